# revision 1
# baseline (speedup 1.0000x reference)
"""Causal self-attention on 8 TRN2 NeuronCores — v3 (pipelined, bf16).

Problem: x[2,2048,1024], wq/wk/wv/wo[1024,1024] (nn.Linear convention,
out = y @ W.T), H=16 heads, D=64, causal softmax, f32.

Sharding: tensor-parallel over heads x data-parallel over batch.
Core i handles batch b=i//4 and head group g=i%4 (4 heads each);
each core returns an f16 partial output projection and the host sums
the 4 partials per batch in f32.

Design: everything bf16 on device; causal mask folded into PSUM by an
identity-matmul pre-write of -1e5 (start=True) that the scores matmul
accumulates onto (start=False), so exp feeds PV directly; attention
runs one head at a time in two query-span-pair passes, ordered
h0A..h3A then h0B..h3B so independent matmul work exists everywhere:
V/QK projections fill the pass-A region, output-projection spans 0/1
fill the pass-B region (their Y rows complete after the A region).
PV runs one ki-step behind scores so ScalarE exp is never gated by
the tensor queue. Softmax 1/sum uses reciprocal_approx_fast on the
DMA-broadcast row (the exact DVE reciprocal on [64,512] costs 3.3us
per call and froze the pipeline in v2). PSUM: 2x mg[128,1024] +
2x pv[65,512] + 2x proj[128,512] = exactly 8 banks.
"""

import sys

for _p in ("/opt/trn_rl_repo", "/root/.axon_site"):
    if _p not in sys.path:
        sys.path.insert(0, _p)

import numpy as np
import ml_dtypes

import concourse.bass as bass
import concourse.mybir as mybir
import concourse.tile as tile
from concourse import bacc
from concourse.bass_utils import run_bass_kernel_spmd

B, T, C, H = 2, 2048, 1024, 16
DH = C // H            # 64 head dim
HG = 4                 # heads per core
GW = HG * DH           # 256 features per head group
NB = T // 128          # 16 key chunks
NS = T // 512          # 4 spans
KC = C // 128          # 8 contraction chunks over C
SCALE = 1.0 / float(np.sqrt(DH))
MASKVAL = -1.0e5       # exp((s+MASKVAL)*SCALE) == 0 for any realistic s
N_CORES = 8

F32 = mybir.dt.float32
F16 = mybir.dt.float16
BF16 = mybir.dt.bfloat16
EXP = mybir.ActivationFunctionType.Exp
COPY = mybir.ActivationFunctionType.Copy


def build_nc():
    nc = bacc.Bacc("TRN2", target_bir_lowering=False, debug=False,
                   num_devices=N_CORES)
    xT = nc.declare_dram_parameter("xT", [C, T], BF16, isOutput=False)
    wqT = nc.declare_dram_parameter("wqT", [C, GW], BF16, isOutput=False)
    wkT = nc.declare_dram_parameter("wkT", [C, GW], BF16, isOutput=False)
    wvT = nc.declare_dram_parameter("wvT", [C, GW], BF16, isOutput=False)
    woT = nc.declare_dram_parameter("woT", [GW, C], BF16, isOutput=False)
    outT = nc.declare_dram_parameter("outT", [C, T], F16, isOutput=True)
    s_dram = nc.dram_tensor("s_scratch", [HG, NS, 512], F32)

    with tile.TileContext(nc) as tc:
        with tc.tile_pool(name="pers", bufs=1) as pers, \
             tc.tile_pool(name="PJ", bufs=2, space="PSUM") as PJ, \
             tc.tile_pool(name="MG", bufs=2, space="PSUM") as MG, \
             tc.tile_pool(name="PV", bufs=1, space="PSUM") as PVP, \
             tc.tile_pool(name="PT", bufs=3) as PT, \
             tc.tile_pool(name="NR", bufs=3) as NR, \
             tc.tile_pool(name="OT", bufs=9) as OT:
            # ---- persistent SBUF; DMAs in consumption order.
            # Weights land in one wide tile each (one DMA trigger each);
            # x streams in span-quarters so the first QK group starts
            # after ~1.5MB instead of the full 4MB.
            def load_w(dram, nch, ncol, tag):
                t = pers.tile([128, nch * ncol], BF16, tag=tag, name=tag)
                nc.gpsimd.dma_start(
                    out=t.rearrange("p (k g) -> p k g", g=ncol),
                    in_=dram.rearrange("(k p) g -> p k g", p=128))
                return [t[:, i * ncol:(i + 1) * ncol] for i in range(nch)]

            wk_t = load_w(wkT, KC, GW, "wkall")
            wq_t = load_w(wqT, KC, GW, "wqall")
            xts = [pers.tile([128, T], BF16, tag=f"xT{i}", name=f"xT{i}")
                   for i in range(KC)]
            for s in range(NS):
                cols = slice(s * 512, (s + 1) * 512)
                for i in range(KC):
                    eng = nc.sync if i % 2 == 0 else nc.scalar
                    eng.dma_start(out=xts[i][:, cols],
                                  in_=xT[i * 128:(i + 1) * 128, cols])
            wv_t = load_w(wvT, KC, GW, "wvall")
            wo_t = load_w(woT, 2, C, "woall")

            qts = [pers.tile([128, T], BF16, tag=f"qT{m}", name=f"qT{m}")
                   for m in range(2)]
            kts = [pers.tile([128, T], BF16, tag=f"kT{m}", name=f"kT{m}")
                   for m in range(2)]
            yts = [pers.tile([128, T], BF16, tag=f"yT{m}", name=f"yT{m}")
                   for m in range(2)]
            vts = [pers.tile([128, HG * 65], BF16, tag=f"V{tb}", name=f"V{tb}")
                   for tb in range(NB)]

            # identity (bf16) and causal-mask pre-write tile:
            # maskM[i,j] = MASKVAL where j<i (query j < key i) else 0
            ident = pers.tile([128, 128], BF16, tag="ident", name="ident")
            nc.gpsimd.memset(ident, 1.0)
            nc.gpsimd.affine_select(
                out=ident, in_=ident, compare_op=mybir.AluOpType.is_ge,
                fill=0.0, base=0, pattern=[[1, 128]], channel_multiplier=-1)
            nc.gpsimd.affine_select(
                out=ident, in_=ident, compare_op=mybir.AluOpType.is_ge,
                fill=0.0, base=0, pattern=[[-1, 128]], channel_multiplier=1)
            maskM = pers.tile([128, 128], BF16, tag="maskM", name="maskM")
            nc.gpsimd.memset(maskM, MASKVAL)
            nc.gpsimd.affine_select(
                out=maskM, in_=maskM, compare_op=mybir.AluOpType.is_ge,
                fill=0.0, base=-1, pattern=[[-1, 128]], channel_multiplier=1)
            ones4 = pers.tile([128, 4], BF16, tag="ones4", name="ones4")
            for j in range(4):
                nc.scalar.activation(
                    out=ones4[:, j:j + 1],
                    in_=nc.const_aps.tensor(1.0, [128, 1]), func=COPY)
            # ones columns of the V tiles are static: write them once
            for tb in range(NB):
                nc.vector.tensor_copy(
                    out=vts[tb].rearrange("p (h c) -> p h c", c=65)[:, :, 64],
                    in_=ones4)

            # ---- emission helpers ----
            def qk_group(wt, dst, m, s):
                """One projection accumulation group: dst[:, s*512:...]"""
                ps = PJ.tile([128, 512], F32, tag="pj", name="pj")
                for k in range(KC):
                    nc.tensor.matmul(
                        ps, wt[k][:, m * 128:(m + 1) * 128],
                        xts[k][:, s * 512:(s + 1) * 512],
                        start=(k == 0), stop=(k == KC - 1))
                nc.vector.tensor_copy(
                    out=dst[:, s * 512:(s + 1) * 512], in_=ps)

            def v_group(tb):
                """V for key chunk tb in natural [t, d] layout (strided cast)."""
                vps = PJ.tile([128, 512], F32, tag="pj", name="pj")
                for k in range(KC):
                    nc.tensor.matmul(
                        vps[:, 0:GW], xts[k][:, tb * 128:(tb + 1) * 128],
                        wv_t[k], start=(k == 0), stop=(k == KC - 1))
                nc.vector.tensor_copy(
                    out=vts[tb].rearrange("p (h c) -> p h c", c=65)[:, :, 0:64],
                    in_=vps.rearrange("p (h c) -> p h c", c=64)[:, 0:4, :])

            def op_group(m, gs, use_scalar=False):
                """Output projection for block m, span gs -> OT staging.

                use_scalar routes the PSUM->SBUF cast through ScalarE —
                it is idle after the last exp, while DVE is busy with the
                norm chains and would stall the PJ ring."""
                op = PJ.tile([128, 512], F32, tag="pj", name="pj")
                for j in range(2):
                    nc.tensor.matmul(
                        op, wo_t[j][:, m * 128:(m + 1) * 128],
                        yts[j][:, gs * 512:(gs + 1) * 512],
                        start=(j == 0), stop=(j == 1))
                half = gs // 2
                ot = ot_tiles[m][half]
                if ot is None:
                    ot = OT.tile([128, 1024], F16, tag="ot", name="ot")
                    ot_tiles[m][half] = ot
                if use_scalar:
                    nc.scalar.activation(
                        out=ot[:, (gs % 2) * 512:(gs % 2 + 1) * 512], in_=op,
                        func=COPY)
                else:
                    nc.vector.tensor_copy(
                        out=ot[:, (gs % 2) * 512:(gs % 2 + 1) * 512], in_=op)
                if gs % 2 == 1:
                    nc.sync.dma_start(
                        out=outT[m * 128:(m + 1) * 128,
                                 half * 1024:(half + 1) * 1024],
                        in_=ot)
                    ot_tiles[m][half] = None

            ot_tiles = [[None, None] for _ in range(8)]

            def norm_span(h, gs, pvt):
                """Normalize completed span: yts <- pv[0:64] / rowsum."""
                m, po = h // 2, (h % 2) * 64
                yv = NR.tile([65, 512], F32, tag="yv", name="yv")
                nc.vector.tensor_copy(out=yv, in_=pvt)
                nc.gpsimd.dma_start(out=s_dram[h, gs, :], in_=yv[64:65, :])
                sb = NR.tile([64, 512], F32, tag="sb", name="sb")
                ssl = s_dram[h, gs, :]
                nc.gpsimd.dma_start(
                    out=sb,
                    in_=bass.AP(tensor=ssl.tensor, offset=ssl.offset,
                                ap=[[0, 64]] + list(ssl.ap)))
                rb = NR.tile([64, 512], F32, tag="rb", name="rb")
                nc.vector.reciprocal_approx_fast(out=rb, in_=sb)
                nc.vector.tensor_mul(
                    out=yts[m][po:po + 64, gs * 512:(gs + 1) * 512],
                    in0=yv[0:64, :], in1=rb)

            # ---- attention for one head, one query-span-pair pass.
            # PV trails scores by one ki step so exp never gates the
            # tensor queue (filler + PV(n-1) + S(n+1) run under exp(n)).
            # ki can run descending so the pass ENDS on its widest
            # strokes, keeping the PE dense across pass boundaries
            # (has_written accumulate-where-set / overwrite-where-clear
            # makes narrow-first PV accumulation correct).
            def attn_pass(h, qpass, fillers, descending=True):
                m, po = h // 2, (h % 2) * 64
                qt, kt = qts[m], kts[m]
                qbase = qpass * 1024
                ki_hi = 8 if qpass == 0 else 16
                ki_order = (list(range(ki_hi - 1, -1, -1)) if descending
                            else list(range(ki_hi)))
                pva = PVP.tile([65, 512], F32, tag="pva", name="pva")
                pvb = PVP.tile([65, 512], F32, tag="pvb", name="pvb")
                pv = (pva, pvb)
                pend = None  # (ki, pt) awaiting PV emission

                def emit_pv(ki, pt):
                    for sp in range(2):
                        gs = qpass * 2 + sp
                        last_ki = 4 * gs + 3
                        if ki > last_ki:
                            continue
                        lo = sp * 512
                        l = max(lo, max(0, 128 * ki - qbase))
                        if l >= lo + 512:
                            continue
                        if descending:
                            first = min(last_ki, ki_hi - 1)
                            st, fin = (ki == first), (ki == 0)
                        else:
                            st, fin = (ki == 0), (ki == last_ki)
                        if st and l > lo:
                            # first (narrowest) write must cover the whole
                            # span uniformly: zero the invalid pt columns
                            # and go full width (PSUM has_written regions
                            # must be uniform per instruction)
                            nc.gpsimd.memset(pt[:, lo:l], 0.0)
                            l = lo
                        nc.tensor.matmul(
                            pv[sp][:, l - lo:512],
                            vts[ki][:, h * 65:(h + 1) * 65],
                            pt[:, l:lo + 512],
                            start=st, stop=fin)
                        if fin:
                            norm_span(h, gs, pv[sp])

                for ki in ki_order:
                    kcol = 128 * ki
                    w0 = max(0, kcol - qbase)
                    diag = kcol >= qbase
                    mg = MG.tile([128, 1024], F32, tag="mg", name="mg")
                    if diag:
                        nc.tensor.matmul(mg[:, w0:w0 + 128], ident, maskM,
                                         start=True, stop=False)
                        nc.tensor.matmul(
                            mg[:, w0:w0 + 128],
                            kt[po:po + 64, kcol:kcol + 128],
                            qt[po:po + 64, qbase + w0:qbase + w0 + 128],
                            start=False, stop=True)
                        segs = []
                        a = w0 + 128
                        if a < 512:
                            segs.append((a, 512))
                        if max(a, 512) < 1024:
                            segs.append((max(a, 512), 1024))
                    else:
                        segs = [(0, 512), (512, 1024)]
                    for (lo, hi) in segs:
                        nc.tensor.matmul(
                            mg[:, lo:hi],
                            kt[po:po + 64, kcol:kcol + 128],
                            qt[po:po + 64, qbase + lo:qbase + hi],
                            start=True, stop=True)
                    pt = PT.tile([128, 1024], BF16, tag="pt", name="pt")
                    nc.scalar.activation(out=pt[:, w0:1024], in_=mg[:, w0:1024],
                                         func=EXP, scale=SCALE)
                    if fillers:
                        f = fillers.pop(0)
                        if f is not None:
                            f()
                    if pend is not None:
                        emit_pv(*pend)
                    pend = (ki, pt)
                if pend is not None:
                    emit_pv(*pend)

            # ---- schedule ----
            # QK projections for head pair 0
            for s in range(NS):
                qk_group(wk_t, kts[0], 0, s)
            for s in range(NS):
                qk_group(wq_t, qts[0], 0, s)
            v_group(0)
            v_group(1)

            # pass-A region fillers. Dependencies: h0-A consumes V(ki) at
            # step ki+1 so V(2..7) must ride h0-A itself; QK for head
            # pair 1 must complete before h2-A's first scores matmul;
            # V(8..15) is only consumed in the pass-B region.
            fa_lists = [
                [(lambda tb: (lambda: v_group(tb)))(tb) for tb in range(2, 8)],
                [(lambda s: (lambda: qk_group(wk_t, kts[1], 1, s)))(s)
                 for s in range(NS)] +
                [(lambda s: (lambda: qk_group(wq_t, qts[1], 1, s)))(s)
                 for s in range(NS)],
                [(lambda tb: (lambda: v_group(tb)))(tb) for tb in range(8, 12)],
                [(lambda tb: (lambda: v_group(tb)))(tb) for tb in range(12, 16)],
            ]

            # h0-A ascending: its own V fillers feed its PV steps
            attn_pass(0, 0, fa_lists[0], descending=False)
            for h in range(1, HG):
                attn_pass(h, 0, fa_lists[h])

            # pass-B region fillers: out-proj spans 0,1, split into 1-MM
            # sub-fillers so a filler step stays under the exp period
            def op_sub_fillers(m, gs):
                cell = {}

                def a():
                    op = PJ.tile([128, 512], F32, tag="pj", name="pj")
                    nc.tensor.matmul(
                        op, wo_t[0][:, m * 128:(m + 1) * 128],
                        yts[0][:, gs * 512:(gs + 1) * 512],
                        start=True, stop=False)
                    cell["op"] = op

                def b():
                    op = cell["op"]
                    nc.tensor.matmul(
                        op, wo_t[1][:, m * 128:(m + 1) * 128],
                        yts[1][:, gs * 512:(gs + 1) * 512],
                        start=False, stop=True)
                    half = gs // 2
                    ot = ot_tiles[m][half]
                    if ot is None:
                        ot = OT.tile([128, 1024], F16, tag="ot", name="ot")
                        ot_tiles[m][half] = ot
                    nc.vector.tensor_copy(
                        out=ot[:, (gs % 2) * 512:(gs % 2 + 1) * 512], in_=op)
                    if gs % 2 == 1:
                        nc.sync.dma_start(
                            out=outT[m * 128:(m + 1) * 128,
                                     half * 1024:(half + 1) * 1024],
                            in_=ot)
                        ot_tiles[m][half] = None

                return [a, b]

            fb = []
            for m in range(8):
                for gs in range(2):
                    fb += op_sub_fillers(m, gs)
            # one sub-filler every other step so filler steps stay light
            fb_lists = []
            for i in range(HG):
                sub = []
                for f in fb[i * 8:(i + 1) * 8]:
                    sub += [f, None]
                fb_lists.append(sub)

            # interleave: a pass consumes its filler list one per ki step.
            # h3-B runs ascending so span 2 finalizes at ki=11 — its norm
            # chain completes while ki 12..15 run, and the tail out-proj
            # starts immediately instead of idling the PE behind the last
            # norm (which is what made the v3/v4 tail ~25us).
            attn_pass(0, 1, fb_lists[0])
            attn_pass(1, 1, fb_lists[1])
            attn_pass(2, 1, fb_lists[2])
            attn_pass(3, 1, fb_lists[3], descending=False)

            # tail: all span-2 groups first (their norms are done well
            # before the last head's span-3 chain), then span 3 — the
            # per-m interleave head-of-line blocked on the final norm.
            for m in range(8):
                op_group(m, 2, use_scalar=True)
            for m in range(8):
                op_group(m, 3, use_scalar=True)
    nc.compile()
    return nc


_NC_CACHE = None


def _get_nc():
    global _NC_CACHE
    if _NC_CACHE is None:
        _NC_CACHE = build_nc()
    return _NC_CACHE


def make_in_maps(x, wq, wk, wv, wo):
    BF = ml_dtypes.bfloat16
    x = np.asarray(x, dtype=np.float32)
    wq = np.asarray(wq, dtype=np.float32)
    wk = np.asarray(wk, dtype=np.float32)
    wv = np.asarray(wv, dtype=np.float32)
    wo = np.asarray(wo, dtype=np.float32)
    in_maps = []
    for core in range(N_CORES):
        b, g = core // HG, core % HG
        rows = slice(g * GW, (g + 1) * GW)
        in_maps.append({
            "xT": np.ascontiguousarray(x[b].T).astype(BF),
            "wqT": np.ascontiguousarray(wq[rows, :].T).astype(BF),
            "wkT": np.ascontiguousarray(wk[rows, :].T).astype(BF),
            "wvT": np.ascontiguousarray(wv[rows, :].T).astype(BF),
            "woT": np.ascontiguousarray(wo[:, rows].T).astype(BF),
        })
    return in_maps


def run(x, wq, wk, wv, wo, trace=False, tmpdir=None):
    nc = _get_nc()
    in_maps = make_in_maps(x, wq, wk, wv, wo)
    res = run_bass_kernel_spmd(nc, in_maps, core_ids=list(range(N_CORES)),
                               trace=trace, tmpdir=tmpdir)
    out = np.zeros((B, T, C), dtype=np.float32)
    for core in range(N_CORES):
        out[core // HG] += res.results[core]["outT"].T.astype(np.float32)
    return out, res


def kernel(x, wq, wk, wv, wo):
    out, _ = run(x, wq, wk, wv, wo)
    return out



# revision 7
# speedup vs baseline: 1.1935x; 1.1935x over previous
"""Causal self-attention on 8 TRN2 NeuronCores — v4 (paired heads, span-major).

Problem: x[2,2048,1024], wq/wk/wv/wo[1024,1024] (nn.Linear convention,
out = y @ W.T), H=16 heads, D=64, causal softmax, f32 I/O.

Sharding: tensor-parallel over heads x data-parallel over batch.
Core i handles batch b=i//4 and head group g=i%4 (4 heads each);
each core returns an f16 partial output projection and the host sums
the 4 partials per batch in f32.

v4 over v3:
- Scores for a head PAIR co-run in the PE array: head-even (kt/qt
  partitions 0-63) at tile_position (0,0), head-odd (64-127) at
  (64,0).  Two K=64 matmuls in disjoint row halves execute
  concurrently, halving scores PE time.
- Span-major attention loop: per (pair, span of 512 queries), ki
  ascending.  Every PSUM accumulation group starts full-width so no
  has_written memset tricks are needed.
- No mask matmuls: diagonal blocks get a triangular bf16 0/1 tile
  multiplied into pt on DVE after exp.
- One exp call per step covers both heads ([128, 1024] PSUM read),
  fewer + wider ACTIVATEs than v3.
- Rowsum broadcast for free: V tiles carry 64 ones-columns, so the PV
  matmul writes the rowsum replicated across PSUM partitions 64-127.
  Norm = reciprocal_approx_fast + tensor_mul on DVE, no DRAM round
  trip (v3's s_dram bounce is gone).
"""

import sys

for _p in ("/opt/trn_rl_repo", "/root/.axon_site"):
    if _p not in sys.path:
        sys.path.insert(0, _p)

import numpy as np
import ml_dtypes

import concourse.bass as bass
import concourse.mybir as mybir
import concourse.tile as tile
from concourse import bacc
from concourse.bass_utils import run_bass_kernel_spmd

B, T, C, H = 2, 2048, 1024, 16
DH = C // H            # 64 head dim
HG = 4                 # heads per core
GW = HG * DH           # 256 features per head group
NB = T // 128          # 16 key chunks
NS = T // 512          # 4 query spans
KC = C // 128          # 8 contraction chunks over C
SCALE = 1.0 / float(np.sqrt(DH))
N_CORES = 8

F32 = mybir.dt.float32
F16 = mybir.dt.float16
BF16 = mybir.dt.bfloat16
EXP = mybir.ActivationFunctionType.Exp
COPY = mybir.ActivationFunctionType.Copy


def build_nc():
    nc = bacc.Bacc("TRN2", target_bir_lowering=False, debug=False,
                   num_devices=N_CORES)
    xT = nc.declare_dram_parameter("xT", [C, T], BF16, isOutput=False)
    wqT = nc.declare_dram_parameter("wqT", [C, GW], BF16, isOutput=False)
    wkT = nc.declare_dram_parameter("wkT", [C, GW], BF16, isOutput=False)
    wvT = nc.declare_dram_parameter("wvT", [C, GW], BF16, isOutput=False)
    woT = nc.declare_dram_parameter("woT", [GW, C], BF16, isOutput=False)
    outT = nc.declare_dram_parameter("outT", [C, T], F16, isOutput=True)

    with tile.TileContext(nc) as tc:
        with tc.tile_pool(name="pers", bufs=1) as pers, \
             tc.tile_pool(name="MG", bufs=2, space="PSUM") as MG, \
             tc.tile_pool(name="PV", bufs=1, space="PSUM") as PVP, \
             tc.tile_pool(name="PJ", bufs=2, space="PSUM") as PJ, \
             tc.tile_pool(name="PT", bufs=3) as PT, \
             tc.tile_pool(name="NR", bufs=4) as NR, \
             tc.tile_pool(name="OT", bufs=6) as OT:
            # ---- persistent SBUF; DMAs in strict consumption-priority
            # order, ALL on the sync queue (hardware-dynamic path): the
            # DMA engines drain descriptors roughly in issue order, so
            # emission order = arrival order.  Weights for pair-0 go
            # first (they gate the very first matmul), then x span 0,
            # then the rest interleaved.
            def load_w(dram, cols, nch, ncol, tag):
                t = pers.tile([128, nch * ncol], BF16, tag=tag, name=tag)
                nc.sync.dma_start(
                    out=t.rearrange("p (k g) -> p k g", g=ncol),
                    in_=dram[:, cols].rearrange("(k p) g -> p k g", p=128))
                return [t[:, i * ncol:(i + 1) * ncol] for i in range(nch)]

            xall = pers.tile([128, KC * T], BF16, tag="xall", name="xall")
            xv = xall.rearrange("p (k t) -> p k t", t=T)
            xts = [xall[:, k * T:(k + 1) * T] for k in range(KC)]
            xTv = xT.rearrange("(k p) t -> p k t", p=128)

            def load_x(ks, s):
                cols = slice(s * 512, (s + 1) * 512)
                nc.sync.dma_start(out=xv[:, ks, cols], in_=xTv[:, ks, cols])

            wk_p = [None, None]
            wq_p = [None, None]
            wk_p[0] = load_w(wkT, slice(0, 128), KC, 128, "wk0")
            wq_p[0] = load_w(wqT, slice(0, 128), KC, 128, "wq0")
            load_x(slice(0, 4), 0)
            load_x(slice(4, 8), 0)
            wv_t = load_w(wvT, slice(0, GW), KC, GW, "wvall")
            load_x(slice(0, 8), 1)
            wk_p[1] = load_w(wkT, slice(128, 256), KC, 128, "wk1")
            wq_p[1] = load_w(wqT, slice(128, 256), KC, 128, "wq1")
            load_x(slice(0, 8), 2)
            wo_t = load_w(woT, slice(0, C), 2, C, "woall")
            load_x(slice(0, 8), 3)

            qts = [pers.tile([128, T], BF16, tag=f"qT{m}", name=f"qT{m}")
                   for m in range(2)]
            kts = [pers.tile([128, T], BF16, tag=f"kT{m}", name=f"kT{m}")
                   for m in range(2)]
            yts = [pers.tile([128, T], BF16, tag=f"yT{m}", name=f"yT{m}")
                   for m in range(2)]
            # V tiles: per head 128 cols = 64 V dims + 64 ones.  The ones
            # columns make the PV matmul write the rowsum replicated on
            # PSUM partitions 64..127 (free cross-partition broadcast).
            vts = [pers.tile([128, HG * 128], BF16, tag=f"V{tb}",
                             name=f"V{tb}") for tb in range(NB)]
            for tb in range(NB):
                nc.vector.memset(
                    vts[tb].rearrange("p (h c) -> p h c", c=128)[:, :, 64:128],
                    1.0)

            # triangular keep-mask: tri[p, c] = 1 where c >= p else 0
            tri = pers.tile([128, 128], BF16, tag="tri", name="tri")
            nc.gpsimd.memset(tri, 1.0)
            nc.gpsimd.affine_select(
                out=tri, in_=tri, compare_op=mybir.AluOpType.is_ge,
                fill=0.0, base=0, pattern=[[1, 128]], channel_multiplier=-1)

            # ---- emission helpers ----
            def qk_group(which, m, s):
                wt = wk_p[m] if which == "k" else wq_p[m]
                dst = kts[m] if which == "k" else qts[m]
                ps = PJ.tile([128, 512], F32, tag="pj", name="pj")
                for k in range(KC):
                    nc.tensor.matmul(
                        ps, wt[k], xts[k][:, s * 512:(s + 1) * 512],
                        start=(k == 0), stop=(k == KC - 1))
                nc.vector.tensor_copy(
                    out=dst[:, s * 512:(s + 1) * 512], in_=ps)

            def v_group(tb):
                vps = PJ.tile([128, 512], F32, tag="pj", name="pj")
                for k in range(KC):
                    nc.tensor.matmul(
                        vps[:, 0:GW], xts[k][:, tb * 128:(tb + 1) * 128],
                        wv_t[k], start=(k == 0), stop=(k == KC - 1))
                nc.vector.tensor_copy(
                    out=vts[tb].rearrange("p (h c) -> p h c", c=128)[:, :, 0:64],
                    in_=vps.rearrange("p (h c) -> p h c", c=64)[:, 0:4, :])

            def op_group(m, s, use_scalar=False):
                op = PJ.tile([128, 512], F32, tag="pj", name="pj")
                for j in range(2):
                    nc.tensor.matmul(
                        op, wo_t[j][:, m * 128:(m + 1) * 128],
                        yts[j][:, s * 512:(s + 1) * 512],
                        start=(j == 0), stop=(j == 1))
                ot = OT.tile([128, 512], F16, tag="ot", name="ot")
                if use_scalar:
                    nc.scalar.activation(out=ot, in_=op, func=COPY)
                else:
                    nc.vector.tensor_copy(out=ot, in_=op)
                nc.sync.dma_start(
                    out=outT[m * 128:(m + 1) * 128, s * 512:(s + 1) * 512],
                    in_=ot)

            def norm(P, par, pv, s):
                """yts[P][par*64 : +64, span s] = pv[0:64] / rowsum.

                pv[64:128] holds the rowsum replicated by the V ones
                columns.  HW custom-DVE ops ignore the input AP's base
                partition (reciprocal with in_ at base 64 silently reads
                base 0 — measured), so every DVE op here keeps its input
                at base 0: CAST the whole pv to SBUF, shift rows 64-127
                down to 0-63 with an SBUF-to-SBUF DMA (address-based, so
                partition moves are fine), then recip + mul at base 0."""
                po = par * 64
                yv = NR.tile([128, 512], F32, tag="yv", name="yv")
                nc.vector.tensor_copy(out=yv, in_=pv)
                sh = NR.tile([64, 512], F32, tag="sh", name="sh")
                nc.gpsimd.dma_start(out=sh, in_=yv[64:128, :])
                rb = NR.tile([64, 512], F32, tag="rb", name="rb")
                nc.vector.reciprocal_approx_fast(out=rb, in_=sh)
                nc.vector.tensor_mul(
                    out=yts[P][po:po + 64, s * 512:(s + 1) * 512],
                    in0=yv[0:64, :], in1=rb)

            # ---- attention region for (pair P, span s).
            # ki ascending 0..4s+3; last 4 ki are diagonal blocks.
            # PV trails by one step so exp never gates the PE queue.
            def attn_region(P, s, fillers):
                kt, qt = kts[P], qts[P]
                qlo = s * 512
                nki = 4 * s + 4
                pvE = PVP.tile([128, 512], F32, tag="pvE", name="pvE")
                pvO = PVP.tile([128, 512], F32, tag="pvO", name="pvO")
                pend = None

                hE, hO = 2 * P, 2 * P + 1

                def emit_pv(ki, w0, pt):
                    st, fin = (ki == 0), (ki == nki - 1)
                    nc.tensor.matmul(
                        pvE[:, w0:512],
                        vts[ki][:, hE * 128:hE * 128 + 128],
                        pt[:, w0:512], start=st, stop=fin)
                    if fin:
                        norm(P, 0, pvE, s)
                    nc.tensor.matmul(
                        pvO[:, w0:512],
                        vts[ki][:, hO * 128:hO * 128 + 128],
                        pt[:, 512 + w0:1024], start=st, stop=fin)
                    if fin:
                        norm(P, 1, pvO, s)

                for ki in range(nki):
                    diag = ki >= 4 * s
                    w0 = 128 * (ki - 4 * s) if diag else 0
                    mg = MG.tile([128, 1024], F32, tag="mg", name="mg")
                    nc.tensor.matmul(
                        mg[:, w0:512],
                        kt[0:64, ki * 128:(ki + 1) * 128],
                        qt[0:64, qlo + w0:qlo + 512],
                        start=True, stop=True)
                    # head-odd writes full width even on diagonal steps so
                    # the pair-wide exp below reads only freshly-written
                    # PSUM; the invalid prefix cols are never read by PV.
                    nc.tensor.matmul(
                        mg[:, 512:1024],
                        kt[64:128, ki * 128:(ki + 1) * 128],
                        qt[64:128, qlo:qlo + 512],
                        start=True, stop=True)
                    pt = PT.tile([128, 1024], BF16, tag="pt", name="pt")
                    nc.scalar.activation(out=pt[:, w0:1024], in_=mg[:, w0:1024],
                                         func=EXP, scale=SCALE)
                    if diag:
                        nc.vector.tensor_mul(
                            out=pt[:, w0:w0 + 128],
                            in0=pt[:, w0:w0 + 128], in1=tri)
                        nc.vector.tensor_mul(
                            out=pt[:, 512 + w0:512 + w0 + 128],
                            in0=pt[:, 512 + w0:512 + w0 + 128], in1=tri)
                    if fillers:
                        f = fillers.pop(0)
                        if f is not None:
                            f()
                    if pend is not None:
                        emit_pv(*pend)
                    pend = (ki, w0, pt)
                emit_pv(*pend)

                # ---- head of the vts ones-slice is static; the V data
                # columns get filled by v_group fillers.

            # ---- schedule ----
            def KQ(which, m, s):
                return lambda: qk_group(which, m, s)

            def VG(tb):
                return lambda: v_group(tb)

            def OPG(m, s):
                return lambda: op_group(m, s, use_scalar=(m % 2 == 1))

            # startup: pair-0 span-0 projections + V0-3 directly
            qk_group("k", 0, 0)
            qk_group("q", 0, 0)
            for tb in range(4):
                v_group(tb)

            fill = {
                (0, 0): [KQ("k", 1, 0), KQ("q", 1, 0)],
                (1, 0): [KQ("k", 0, 1), KQ("q", 0, 1), VG(4), VG(5)],
                (0, 1): [VG(6), VG(7), KQ("k", 1, 1), KQ("q", 1, 1),
                         OPG(0, 0), OPG(1, 0), OPG(2, 0), OPG(3, 0)],
                (1, 1): [KQ("k", 0, 2), KQ("q", 0, 2), VG(8), VG(9),
                         OPG(4, 0), OPG(5, 0), OPG(6, 0), OPG(7, 0)],
                (0, 2): [VG(10), VG(11), KQ("k", 1, 2), KQ("q", 1, 2)] +
                        [OPG(m, 1) for m in range(8)],
                (1, 2): [KQ("k", 0, 3), KQ("q", 0, 3),
                         VG(12), VG(13), VG(14), VG(15)],
                (0, 3): [KQ("k", 1, 3), KQ("q", 1, 3)] +
                        [OPG(m, 2) for m in range(4)],
                (1, 3): [OPG(m, 2) for m in range(4, 8)],
            }
            for s in range(NS):
                for P in range(2):
                    attn_region(P, s, fill[(P, s)])

            # tail: final span out-projections
            for m in range(8):
                op_group(m, 3, use_scalar=(m % 2 == 1))
    nc.compile()
    return nc


_NC_CACHE = None


def _get_nc():
    global _NC_CACHE
    if _NC_CACHE is None:
        _NC_CACHE = build_nc()
    return _NC_CACHE


def make_in_maps(x, wq, wk, wv, wo):
    BF = ml_dtypes.bfloat16
    x = np.asarray(x, dtype=np.float32)
    wq = np.asarray(wq, dtype=np.float32)
    wk = np.asarray(wk, dtype=np.float32)
    wv = np.asarray(wv, dtype=np.float32)
    wo = np.asarray(wo, dtype=np.float32)
    in_maps = []
    for core in range(N_CORES):
        b, g = core // HG, core % HG
        rows = slice(g * GW, (g + 1) * GW)
        in_maps.append({
            "xT": np.ascontiguousarray(x[b].T).astype(BF),
            "wqT": np.ascontiguousarray(wq[rows, :].T).astype(BF),
            "wkT": np.ascontiguousarray(wk[rows, :].T).astype(BF),
            "wvT": np.ascontiguousarray(wv[rows, :].T).astype(BF),
            "woT": np.ascontiguousarray(wo[:, rows].T).astype(BF),
        })
    return in_maps


def run(x, wq, wk, wv, wo, trace=False, tmpdir=None):
    nc = _get_nc()
    in_maps = make_in_maps(x, wq, wk, wv, wo)
    res = run_bass_kernel_spmd(nc, in_maps, core_ids=list(range(N_CORES)),
                               trace=trace, tmpdir=tmpdir)
    out = np.zeros((B, T, C), dtype=np.float32)
    for core in range(N_CORES):
        out[core // HG] += res.results[core]["outT"].T.astype(np.float32)
    return out, res


def kernel(x, wq, wk, wv, wo):
    out, _ = run(x, wq, wk, wv, wo)
    return out
